# revision 22
# baseline (speedup 1.0000x reference)
"""Trainium2 kernel for ChannelQuadLayer.

Per-pixel quadratic channel expansion + 1x1 conv:
    quad = x[:, ii] * x[:, jj]  (all 2080 upper-tri channel pairs)
    y    = concat([x, quad])    -> [B, 2144, H, W]
    out  = einsum('bchw,oc->bohw', y, fc_w)

Strategy (8 NeuronCores, batch-parallel, one sample per core):
  * The 2080 unordered channel pairs are exactly the cyclic diagonals
    d=0..32 of the 64-channel index ring: pairs {i, (i+d)%64}.
  * Host prepares 9 "rotation buffers" B_k = [roll(x,-t_k); roll(x,-u_k)]
    (128 partitions x 4096 pixels, bf16). A single elementwise multiply
    of two such buffers yields TWO complete cyclic diagonals. A
    difference cover produces all diagonals 1..32 in 16 multiplies;
    diagonal 0 (squares) comes from one Square op.
  * y-rows: 64 linear + 64 squares + 16*128 pair rows = 2176 = 17*128,
    an exact 17-chunk contraction. fc_w is permuted/padded to this row
    order on the host (duplicate pair rows get zero weight).
  * Everything on-chip is bf16 (except fp32 PSUM accumulation): bf16
    matmul runs at the same 1 row/cycle as fp32r, but halves DMA bytes
    and doubles VectorE multiply throughput (2x_1p mode).
  * GEMM: out[256, 4096] = Wt[2176, 256]^T @ y[2176, 4096] on TensorE,
    accumulating 17 chunks into PSUM, k-outer so each y chunk is
    consumed right after its producer.

DMA/engine-stream discipline (from hardware traces; 16 DMA engines
stripe every transfer 8 rows each and round-robin the queue classes,
~25GB/s per engine while busy, completion semaphores land ~2.3us after
the data; per-pass 1024px slices beat both tiny slices, which starve
TensorE chunk-by-chunk, and monolithic whole-buffer loads, which stall
the next pass for tens of us):
  * Pass 0's eight b slices are split over the sync and gpsimd queues
    (DMA can only be issued from SP/Act/GpSimd); later passes use
    sync+scalar so gpsimd never blocks and DVE only multiplies.
  * All 17 per-chunk weight tiles ride the scalar queue in KORD order
    (consumption order = the order operand buffers land); a weight
    chunk queued behind a big b transfer once arrived at 27us and
    stalled TensorE for 1.4us.
  * Chunk 0 ([x; x^2], from the single earliest buffer) is consumed
    first so TensorE starts before any buffer PAIR has landed.
  * PSUM drains may only run after a pass's final matmul, so they are
    emitted one pass late, behind the next pass's y0 ACTIVATEs, and
    the final pass splits its drain across Act+DVE / scalar+sync.
  * Dummy matmuls on a memset tile keep the PE p-state (needs ~3us of
    continuous work for max clock) warm through the initial DMA window;
    trimming them lets the clock sag and measurably hurts.
  * Output is staged to bf16 and DMA'd as bf16; host converts to fp32.
"""

import sys

sys.path.insert(0, "/opt/trn_rl_repo")

import numpy as np

import concourse.bass as bass
import concourse.tile as tile
from concourse import bacc, mybir
from concourse.bass_utils import run_bass_kernel_spmd

B, C, H, W = 8, 64, 64, 64
PIX = H * W  # 4096
OUT = 256
NCORES = 8

# rotation difference cover: ops (i,j) give diagonals D(t_j-t_i) (top half)
# and D(u_j-u_i) (bottom half); together exactly {1..32}. 8 buffers
# (found by annealing over C(8,2) pairs + rotations) instead of the
# previous 9: one less 1MB/core input transfer chain.
T_ROT = [0, 46, 51, 19, 10, 48, 4, 43]
U_ROT = [0, 45, 7, 46, 34, 16, 42, 48]
OPS = [(0, 1), (0, 2), (0, 4), (0, 7), (1, 3), (1, 4), (1, 6), (2, 6),
       (2, 7), (3, 4), (3, 6), (3, 7), (4, 7), (5, 6), (5, 7), (6, 7)]
NB = len(T_ROT)        # 9 rotation buffers
KCH = 1 + len(OPS)     # 17 contraction chunks of 128 rows
# chunk consumption order = order their operand buffers land, given the
# b-load queue schedules below
KORD = [0, 1, 2, 4, 9, 3, 6, 13, 7, 8, 16, 5, 10, 11, 12, 14, 15]
B_SYNC = [0, 2, 7, 6]      # sync-queue b loads (first-use order)
B_AUX = [1, 4, 3, 5]       # gpsimd (pass 0) / scalar (later passes)
PASS_FD = [1024, 1024, 1024, 768, 256]
assert sum(PASS_FD) == PIX
NPASS = len(PASS_FD)
N_WARM = 22                # dummy matmuls to pre-ramp the PE clock

F32 = mybir.dt.float32
BF16 = mybir.dt.bfloat16
NP_BF16 = mybir.dt.np(BF16)


def row_pairs():
    """Channel pair (c1, c2) for every global y row, or ('lin', c)."""
    rows = []
    for p in range(128):  # chunk 0
        rows.append(("lin", p) if p < 64 else (p - 64, p - 64))
    for (i, j) in OPS:
        for p in range(128):
            if p < 64:
                c1, c2 = (p + T_ROT[i]) % 64, (p + T_ROT[j]) % 64
            else:
                c1, c2 = (p - 64 + U_ROT[i]) % 64, (p - 64 + U_ROT[j]) % 64
            rows.append((min(c1, c2), max(c1, c2)))
    return rows


def build_wt(fc_w):
    """Permute fc_w [OUT, 2144] into Wt [KCH, 128, OUT] matching y rows."""
    ii, jj = np.triu_indices(C)
    pair2col = {(a, b): C + k for k, (a, b) in enumerate(zip(ii, jj))}
    wt = np.zeros((KCH * 128, OUT), np.float32)
    seen = set()
    for g, r in enumerate(row_pairs()):
        if r[0] == "lin":
            wt[g] = fc_w[:, r[1]]
        elif r not in seen:
            seen.add(r)
            wt[g] = fc_w[:, pair2col[r]]
    assert len(seen) == C * (C + 1) // 2
    return np.ascontiguousarray(wt.reshape(KCH, 128, OUT))


def widths(FD):
    """Split a pass width into matmul tiles of <=512 (PSUM bank limit)."""
    ws = [512] * (FD // 512)
    if FD % 512:
        ws.append(FD % 512)
    return ws


_cached = None


def _build_module():
    global _cached
    if _cached is not None:
        return _cached
    nc = bacc.Bacc("TRN2", target_bir_lowering=False, debug=False,
                   num_devices=NCORES)
    b_d = [nc.dram_tensor(f"b{i}", [128, PIX], BF16, kind="ExternalInput")
           for i in range(NB)]
    # weight matrix, partition-major; column block k*OUT:(k+1)*OUT is chunk k
    wt_d = nc.dram_tensor("wt", [128, KCH * OUT], BF16, kind="ExternalInput")
    out_d = nc.dram_tensor("out", [2, 128, PIX], BF16, kind="ExternalOutput")

    with tile.TileContext(nc) as tc:
        with tc.tile_pool(name="wt", bufs=1) as wt_pool, \
             tc.tile_pool(name="bsrc", bufs=2) as b_pool, \
             tc.tile_pool(name="y", bufs=8) as y_pool, \
             tc.tile_pool(name="ostage", bufs=4) as o_pool, \
             tc.tile_pool(name="psum", bufs=8, space="PSUM") as ps_pool:

            # PE clock warmup: memset a scratch tile, then dummy matmuls
            # while the first DMAs are in flight. The scratch PSUM bank
            # is reused by a later pass long after.
            warm = wt_pool.tile([128, 512], BF16, name="warm")
            nc.gpsimd.memset(warm, 0)
            wps = ps_pool.tile([128, 512], F32, tag="ps", name="warm_ps")
            for _ in range(N_WARM):
                nc.tensor.matmul(wps[:, :512], warm[:, :128], warm[:, :512],
                                 start=True, stop=True, skip_group_check=True)

            # 17 per-chunk weight tiles: the first matmul depends only on
            # the tiny chunk transfer it actually reads.
            wt_t = [wt_pool.tile([128, OUT], BF16, name=f"wt{k}")
                    for k in range(KCH)]

            def emit_drain(psum, ws, FD, off, ps, split):
                """Stage PSUM to bf16 and DMA out. split=True puts m=1 on
                DVE+sync (parallel tail chains for the final pass);
                otherwise everything runs on scalar."""
                NT = len(ws)
                for m in range(2):
                    ot = o_pool.tile([128, 1024], BF16, tag="ostage",
                                     name=f"o{ps}_{m}")
                    for n, w in enumerate(ws):
                        src = psum[m * NT + n][:, :w]
                        dst = ot[:, n * 512:n * 512 + w]
                        if split and m == 1:
                            nc.vector.tensor_copy(dst, src)
                        else:
                            nc.scalar.activation(
                                dst, src, mybir.ActivationFunctionType.Identity)
                    eng = nc.sync if (split and m == 1) else nc.scalar
                    eng.dma_start(out_d.ap()[m, :, off:off + FD], ot[:, :FD])

            pending = None  # previous pass's drain, emitted after this
                            # pass's y0 so the scalar queue stays in order
            off = 0
            for ps, FD in enumerate(PASS_FD):
                ws = widths(FD)
                NT = len(ws)
                last = ps == NPASS - 1
                bt = [None] * NB
                aux = nc.gpsimd if ps == 0 else nc.scalar
                for eng, order in [(nc.sync, B_SYNC), (aux, B_AUX)]:
                    for i in order:
                        t = b_pool.tile([128, 1024], BF16, tag=f"b{i}",
                                        name=f"b{i}_{ps}")
                        if ps == 0 and i in (0, 1):
                            # split the first buffer on each queue so the
                            # first chunks' first halves unblock a
                            # half-transfer earlier
                            eng.dma_start(t[:, :512], b_d[i].ap()[:, :512])
                            eng.dma_start(t[:, 512:FD],
                                          b_d[i].ap()[:, 512:FD])
                        else:
                            eng.dma_start(t[:, :FD],
                                          b_d[i].ap()[:, off:off + FD])
                        bt[i] = t
                if ps == 0:
                    # all weight chunks on the scalar queue (it carries
                    # nothing else in pass 0), consumption order
                    for k in KORD:
                        nc.scalar.dma_start(
                            wt_t[k], wt_d.ap()[:, k * OUT:(k + 1) * OUT])

                psum = [ps_pool.tile([128, 512], F32, tag="ps",
                                     name=f"ps{ps}_{g}")
                        for g in range(2 * NT)]

                for ki, k in enumerate(KORD):
                    yk = y_pool.tile([128, 1024], BF16, tag="y",
                                     name=f"y{ps}_{k}")
                    if k == 0:
                        # linear rows + squares from the resident b0 tile;
                        # passes 0-1 use DVE (the scalar queue is still
                        # issuing the 17 weight transfers at kernel start)
                        if ps == 0:
                            for lo, hi in ((0, 512), (512, FD)):
                                nc.vector.tensor_copy(yk[0:64, lo:hi],
                                                      bt[0][0:64, lo:hi])
                                nc.vector.tensor_mul(yk[64:128, lo:hi],
                                                     bt[0][64:128, lo:hi],
                                                     bt[0][64:128, lo:hi])
                        elif ps == 1:
                            nc.vector.tensor_copy(yk[0:64, :FD],
                                                  bt[0][0:64, :FD])
                            nc.vector.tensor_mul(yk[64:128, :FD],
                                                 bt[0][64:128, :FD],
                                                 bt[0][64:128, :FD])
                        else:
                            nc.scalar.activation(
                                yk[0:64, :FD], bt[0][0:64, :FD],
                                mybir.ActivationFunctionType.Identity)
                            nc.scalar.activation(
                                yk[64:128, :FD], bt[0][64:128, :FD],
                                mybir.ActivationFunctionType.Square)
                    elif ps == 0 and k == 1:
                        i, j = OPS[k - 1]
                        for lo, hi in ((0, 512), (512, FD)):
                            nc.vector.tensor_mul(yk[:, lo:hi],
                                                 bt[i][:, lo:hi],
                                                 bt[j][:, lo:hi])
                    else:
                        i, j = OPS[k - 1]
                        nc.vector.tensor_mul(yk[:, :FD], bt[i][:, :FD],
                                             bt[j][:, :FD])
                    for m in range(2):
                        lhsT = wt_t[k][:, m * 128:(m + 1) * 128]
                        for n, w in enumerate(ws):
                            nc.tensor.matmul(
                                psum[m * NT + n][:, :w],
                                lhsT,
                                yk[:, n * 512:n * 512 + w],
                                start=(ki == 0), stop=(ki == KCH - 1))
                    if ki == 1 and pending is not None:
                        # previous pass's drain, now safely behind this
                        # pass's y0 in the scalar stream
                        emit_drain(*pending, split=False)
                        pending = None

                if last:
                    emit_drain(psum, ws, FD, off, ps, split=True)
                else:
                    pending = (psum, ws, FD, off, ps)
                off += FD
    nc.compile()
    _cached = nc
    return nc


def make_in_maps(x, wt):
    # [KCH, 128, OUT] -> [128, KCH*OUT]
    wtp = np.ascontiguousarray(
        wt.transpose(1, 0, 2).reshape(128, KCH * OUT)).astype(NP_BF16)
    in_maps = []
    for b in range(B):
        xc = np.ascontiguousarray(
            np.asarray(x[b], np.float32).reshape(C, PIX)).astype(NP_BF16)
        m = {"wt": wtp}
        for i in range(NB):
            m[f"b{i}"] = np.ascontiguousarray(np.concatenate(
                [np.roll(xc, -T_ROT[i], axis=0), np.roll(xc, -U_ROT[i], axis=0)]))
        in_maps.append(m)
    return in_maps


def assemble_out(res):
    outs = []
    for b in range(B):
        o = np.asarray(res.results[b]["out"], dtype=np.float32)  # [2,128,PIX]
        outs.append(o.reshape(OUT, H, W))
    return np.stack(outs)


def kernel(x, fc_w):
    x = np.asarray(x, dtype=np.float32)
    fc_w = np.asarray(fc_w, dtype=np.float32)
    nc = _build_module()
    wt = build_wt(fc_w)
    res = run_bass_kernel_spmd(nc, make_in_maps(x, wt), list(range(NCORES)))
    return assemble_out(res)
